# revision 18
# baseline (speedup 1.0000x reference)
"""Trainium2 Bass kernel for the VQ-codebook clustering model.

Computes, for x [131072, 784] fp32 and centers [64, 784] fp32:
    logits = 20 * (x @ centers.T - 0.5 * ||centers||^2)
    w      = softmax(logits, axis=1)
    recon  = w @ centers
and returns (recon, x) exactly like the reference (x0 == x here since x is
already 2-D, so it is passed through on the host).

Sharding: pure data parallel — x is split along the batch dim into 8 equal
shards of 16384 rows, centers are replicated; each NeuronCore runs the same
program on its shard and the host concatenates the outputs.

Per-core structure (macro-tile = 512 rows, super-tile = 4 macro-tiles):
  - DMA x in a (p g) layout: partition p holds 4 *consecutive* rows, so each
    partition's DMA piece is 12.5 KB contiguous (the within-group row
    permutation is honored symmetrically by the output store).  Two extra
    ones-columns feed the augmented bias rows.
  - Phase 1 (per super-tile): PE identity-transposes x into 7 feature chunks
    [112/114, 512] in f32r transpose mode; each psum chunk is evicted by a
    DVE half + ACT half in parallel into float32r SBUF tiles.
  - Phase 2 (per super-tile, software-pipelined): mm1 in float32r
    (logitsT [64,512] = sum_c ct_chunk.T @ xT_chunk, stationary = tiny
    centers chunk, 512-wide moving stream); the softmax/mm2 block of
    macro m-1 is emitted between mm1(m) chunks so the PE stream stays dense
    and the HAM clock gate stays at 2.4 GHz.  Chunk 6 carries hi/lo split
    augmented rows of -0.5*SCALE*||c||^2 so FP22 cannot bite the bias.
  - Softmax per 128-row group: PE transposes logitsT back (fp32, exact),
    DVE reduce-max (negated), ACT Exp with per-row bias + fused row-sum,
    DVE reciprocal; PE transposes e; mm2 in float32r against centers in
    natural layout; recon evicted DVE-half/ACT-half with 1/Z folded in.
"""

from contextlib import ExitStack

import numpy as np

import concourse.bass as bass
import concourse.tile as tile
import concourse.mybir as mybir
from concourse import bacc, bass_isa, masks
from concourse.bass_utils import run_bass_kernel_spmd

F32 = mybir.dt.float32
F32R = mybir.dt.float32r
BF16 = mybir.dt.bfloat16

N_CORES = 8
N_ROWS = 131072
D = 784
K = 64
SCALE = 20.0
ROWS_PER_CORE = N_ROWS // N_CORES  # 16384

GROUP = 128                  # rows per psum group (psum partition count)
GROUPS_PER_TILE = 4          # groups per macro tile
TILE_ROWS = GROUP * GROUPS_PER_TILE  # 512
SUPER = 4                    # macro-tiles per phase-batched super-tile
CHUNK = 112                  # feature-chunk width for the contraction
N_CHUNKS = D // CHUNK        # 7
NONES = 2                    # ones columns feeding the two augmented rows
EVICT_SPLIT = 256            # DVE evicts cols [0:256], ACT [256:512]
REC_SPLIT = 384              # recon evict: DVE [0:384], ACT [384:784]


def _pp(c):
    return CHUNK + NONES if c == N_CHUNKS - 1 else CHUNK


def emit_core_program(ctx: ExitStack, tc: tile.TileContext, x_ap, c_ap, y_ap,
                      rows_per_core):
    nc = tc.nc
    n_tiles = rows_per_core // TILE_ROWS

    const = ctx.enter_context(tc.tile_pool(name="const", bufs=1))
    xin_pool = ctx.enter_context(tc.tile_pool(name="xin", bufs=6))
    yout_pool = ctx.enter_context(tc.tile_pool(name="yout", bufs=2))
    xtsb_pool = ctx.enter_context(tc.tile_pool(name="xtsb",
                                               bufs=max(18, SUPER * N_CHUNKS + 4)))
    lt_pool = ctx.enter_context(tc.tile_pool(name="ltsb", bufs=2))
    et_pool = ctx.enter_context(tc.tile_pool(name="etsb", bufs=2))
    gmax_pool = ctx.enter_context(tc.tile_pool(name="gmax", bufs=2))
    small_pool = ctx.enter_context(tc.tile_pool(name="small", bufs=8))

    xtps_pool = ctx.enter_context(tc.tile_pool(name="xtps", bufs=2, space="PSUM"))
    ltps_pool = ctx.enter_context(tc.tile_pool(name="ltps", bufs=3, space="PSUM"))
    rec_pool = ctx.enter_context(tc.tile_pool(name="recps", bufs=1, space="PSUM"))

    # ---- preamble ----------------------------------------------------------
    ident = const.tile([128, 128], F32, tag="ident")
    masks.make_identity(nc, ident[:])
    ident_r = const.tile([128, 128], F32R, tag="identr")
    nc.vector.tensor_copy(ident_r[:], ident[:])

    cen = const.tile([K, D], F32, tag="cen")
    nc.sync.dma_start(out=cen[:], in_=c_ap[:, :])
    # second copy tagged float32r (same bytes) for the fp32r reconstruction
    # matmul — the BIR verifier wants fp32r operands produced as fp32r, and
    # only SWDGE DMA may change the dtype tag.
    cen_r = const.tile([K, D + 2], F32R, tag="cenr")
    nc.gpsimd.dma_start(out=cen_r[:, 0:D], in_=c_ap[:, :])
    nc.gpsimd.memset(cen_r[:, D:D + 2].bitcast(mybir.dt.uint32), 0x3F800000)
    ones1_r = const.tile([1, K], F32R, tag="ones1r")
    nc.gpsimd.memset(ones1_r[:].bitcast(mybir.dt.uint32), 0x3F800000)

    # a = -0.5 * ||c||^2 per center, split into bf16 hi + fp32 lo so the
    # fp32r (FP22) read of the augmented rows cannot lose bias precision.
    sq_scratch = const.tile([K, D], F32, tag="sqscr")
    ssq = const.tile([K, 1], F32, tag="ssq")
    nc.scalar.activation(sq_scratch[:], cen[:],
                         mybir.ActivationFunctionType.Square,
                         accum_out=ssq[:])
    a_full = const.tile([K, 1], F32, tag="afull")
    nc.vector.tensor_scalar_mul(a_full[:], ssq[:], -0.5)
    a_hi16 = const.tile([K, 1], BF16, tag="ahi16")
    nc.vector.tensor_copy(a_hi16[:], a_full[:])
    a_hi = const.tile([K, 1], F32, tag="ahi")
    nc.vector.tensor_copy(a_hi[:], a_hi16[:])
    a_lo = const.tile([K, 1], F32, tag="alo")
    nc.vector.tensor_sub(a_lo[:], a_full[:], a_hi[:])

    # ct[:, 64c:64c+64] = chunk c of (SCALE * centers.T) as float32r;
    # partition rows 112/113 of chunk 6 are the augmented hi/lo bias rows.
    ct = const.tile([CHUNK + NONES, K * N_CHUNKS], F32R, tag="ct")
    for c in range(N_CHUNKS - 1):
        pre_ps = xtps_pool.tile([CHUNK, K], F32, tag="xtps")
        nc.tensor.transpose(out=pre_ps[0:CHUNK, 0:K],
                            in_=cen[:, c * CHUNK:(c + 1) * CHUNK],
                            identity=ident[0:K, 0:K])
        nc.scalar.mul(ct[0:CHUNK, c * K:(c + 1) * K], pre_ps[0:CHUNK, 0:K], SCALE)
    scr6 = const.tile([K, CHUNK + NONES], F32, tag="scr6")
    nc.vector.tensor_copy(scr6[:, 0:CHUNK],
                          cen[:, (N_CHUNKS - 1) * CHUNK:N_CHUNKS * CHUNK])
    nc.vector.tensor_copy(scr6[:, CHUNK:CHUNK + 1], a_hi[:])
    nc.vector.tensor_copy(scr6[:, CHUNK + 1:CHUNK + 2], a_lo[:])
    pre_ps6 = xtps_pool.tile([CHUNK + NONES, K], F32, tag="xtps")
    nc.tensor.transpose(out=pre_ps6[0:CHUNK + NONES, 0:K], in_=scr6[:],
                        identity=ident[0:K, 0:K])
    nc.scalar.mul(ct[0:CHUNK + NONES, (N_CHUNKS - 1) * K:N_CHUNKS * K],
                  pre_ps6[0:CHUNK + NONES, 0:K], SCALE)

    # ---- helpers -----------------------------------------------------------
    def emit_load_and_transpose(t):
        """Phase-1 body for macro-tile t: load + 28 transposes + evictions."""
        x_in = xin_pool.tile([128, GROUPS_PER_TILE, D + NONES], F32R, tag="xin")
        nc.sync.dma_start(
            out=x_in[:, :, 0:D],
            in_=x_ap[t * TILE_ROWS:(t + 1) * TILE_ROWS, :].rearrange(
                "(p g) f -> p g f", g=GROUPS_PER_TILE),
        )
        nc.gpsimd.memset(x_in[:, :, D:D + NONES].bitcast(mybir.dt.uint32),
                         0x3F800000)
        xt_sb = []
        for c in range(N_CHUNKS):
            pp = _pp(c)
            xt_ps = xtps_pool.tile([CHUNK + NONES, TILE_ROWS], F32R, tag="xtps")
            for g in range(GROUPS_PER_TILE):
                nc.tensor.transpose(
                    out=xt_ps[0:pp, g * GROUP:(g + 1) * GROUP],
                    in_=x_in[:, g, c * CHUNK:c * CHUNK + pp],
                    identity=ident_r[:, 0:GROUP])
            sb = xtsb_pool.tile([CHUNK + NONES, TILE_ROWS], F32R, tag="xtsb")
            nc.vector.tensor_copy(sb[0:pp, 0:EVICT_SPLIT],
                                  xt_ps[0:pp, 0:EVICT_SPLIT])
            nc.scalar.copy(sb[0:pp, EVICT_SPLIT:TILE_ROWS],
                           xt_ps[0:pp, EVICT_SPLIT:TILE_ROWS])
            xt_sb.append(sb)
        return xt_sb

    def emit_mm1(xt_sb):
        """Logits-transposed accumulation + transposed-space max subtract.

        After the 7 chunk matmuls, the raw logitsT are evicted, gpsimd
        computes the per-row (cross-partition) max, and a k=1 ones-matmul
        broadcasts -max back into the psum so ACT Exp needs no bias and
        writes eT [64, 512] directly in mm2's operand layout.
        """
        lt_ps = ltps_pool.tile([K, TILE_ROWS], F32, tag="ltps")
        for c in range(N_CHUNKS):
            pp = _pp(c)
            nc.tensor.matmul(out=lt_ps[:, :],
                             lhsT=ct[0:pp, c * K:(c + 1) * K],
                             rhs=xt_sb[c][0:pp, :],
                             start=(c == 0), stop=(c == N_CHUNKS - 1))
        lt_sb = lt_pool.tile([K, TILE_ROWS], F32, tag="ltsb")
        nc.vector.tensor_copy(lt_sb[:, 0:EVICT_SPLIT],
                              lt_ps[:, 0:EVICT_SPLIT])
        nc.scalar.copy(lt_sb[:, EVICT_SPLIT:TILE_ROWS],
                       lt_ps[:, EVICT_SPLIT:TILE_ROWS])
        gmax = gmax_pool.tile([K, TILE_ROWS], F32, tag="gmax")
        nc.gpsimd.partition_all_reduce(gmax[:], lt_sb[:], channels=K,
                                       reduce_op=bass_isa.ReduceOp.max)
        negmax_r = small_pool.tile([1, TILE_ROWS], F32R, tag="negmaxr")
        nc.vector.tensor_scalar_mul(negmax_r[:], gmax[0:1, :], -1.0)
        return lt_ps, negmax_r

    def emit_expT(lt_ps, negmax_r):
        """Stage B: broadcast -max into the open psum group, exponentiate."""
        nc.tensor.matmul(out=lt_ps[:, :], lhsT=ones1_r[:],
                         rhs=negmax_r[:], start=False, stop=True,
                         skip_group_check=True)
        eT_sb = et_pool.tile([K, TILE_ROWS], F32R, tag="etsb")
        nc.scalar.activation(eT_sb[:], lt_ps[:, :],
                             mybir.ActivationFunctionType.Exp)
        return eT_sb

    def emit_softmax_mm2(t, eT_sb):
        """Reconstruction (with fused Z column) + store for one macro-tile."""
        out_sb = yout_pool.tile([128, GROUPS_PER_TILE, D], F32, tag="yout")
        for g in range(GROUPS_PER_TILE):
            lhsT = eT_sb[:, g * GROUP:(g + 1) * GROUP]
            rec_ps = rec_pool.tile([GROUP, D + 2], F32, tag="recps")
            nc.tensor.matmul(out=rec_ps[:, 0:512], lhsT=lhsT,
                             rhs=cen_r[:, 0:512], start=True, stop=True)
            nc.tensor.matmul(out=rec_ps[:, 512:D + 2], lhsT=lhsT,
                             rhs=cen_r[:, 512:D + 2], start=True, stop=True)

            rinv = small_pool.tile([GROUP, 1], F32, tag="rinv")
            nc.vector.reciprocal(rinv[:], rec_ps[:, D:D + 1])

            # evict with 1/Z normalization folded in, split DVE/ACT
            nc.vector.tensor_scalar_mul(out_sb[:, g, 0:REC_SPLIT],
                                        rec_ps[:, 0:REC_SPLIT], rinv[:])
            nc.scalar.mul(out_sb[:, g, REC_SPLIT:D],
                          rec_ps[:, REC_SPLIT:D], rinv[:])

        nc.sync.dma_start(
            out=y_ap[t * TILE_ROWS:(t + 1) * TILE_ROWS, :].rearrange(
                "(p g) f -> p g f", g=GROUPS_PER_TILE),
            in_=out_sb[:],
        )

    # ---- main loop: phase-batched super-tiles ------------------------------
    pend_b = None  # (t, lt_ps, negmax_r): awaiting broadcast+exp
    pend_c = None  # (t, eT_sb): awaiting mm2+store
    for t0 in range(0, n_tiles, SUPER):
        ts = list(range(t0, min(t0 + SUPER, n_tiles)))
        xts = [emit_load_and_transpose(t) for t in ts]
        for i, t in enumerate(ts):
            lt_ps, negmax_r = emit_mm1(xts[i])
            if pend_b is not None:
                tb, blt, bneg = pend_b
                eT = emit_expT(blt, bneg)
                if pend_c is not None:
                    emit_softmax_mm2(*pend_c)
                pend_c = (tb, eT)
            pend_b = (t, lt_ps, negmax_r)
    tb, blt, bneg = pend_b
    eT = emit_expT(blt, bneg)
    if pend_c is not None:
        emit_softmax_mm2(*pend_c)
    emit_softmax_mm2(tb, eT)


def build_kernel(rows_per_core=ROWS_PER_CORE):
    nc = bacc.Bacc("TRN2", target_bir_lowering=False, debug=False)
    x_d = nc.dram_tensor("x", [rows_per_core, D], F32R, kind="ExternalInput")
    c_d = nc.dram_tensor("centers", [K, D], F32, kind="ExternalInput")
    y_d = nc.dram_tensor("y", [rows_per_core, D], F32, kind="ExternalOutput")
    with tile.TileContext(nc) as tc:
        with ExitStack() as ctx:
            emit_core_program(ctx, tc, x_d.ap(), c_d.ap(), y_ap=y_d.ap(),
                              rows_per_core=rows_per_core)
    nc.compile()
    return nc


_NC_CACHE = {}


def _get_nc(rows_per_core=ROWS_PER_CORE):
    if rows_per_core not in _NC_CACHE:
        _NC_CACHE[rows_per_core] = build_kernel(rows_per_core)
    return _NC_CACHE[rows_per_core]


def run_on_cores(x, centers, trace=False, **kwargs):
    """Run the SPMD kernel on 8 cores; returns (recon, BassKernelResults)."""
    x = np.ascontiguousarray(x, dtype=np.float32)
    centers = np.ascontiguousarray(centers, dtype=np.float32)
    assert x.shape == (N_ROWS, D) and centers.shape == (K, D)
    nc = _get_nc()
    shards = x.reshape(N_CORES, ROWS_PER_CORE, D)
    in_maps = [{"x": shards[i], "centers": centers} for i in range(N_CORES)]
    br = run_bass_kernel_spmd(nc, in_maps, list(range(N_CORES)), trace=trace,
                              **kwargs)
    recon = np.concatenate([r["y"] for r in br.results], axis=0)
    return recon, br


def kernel(x, centers):
    x = np.ascontiguousarray(x, dtype=np.float32)
    recon, _ = run_on_cores(x, centers)
    return recon, x


# revision 19
# speedup vs baseline: 1.1572x; 1.1572x over previous
"""Trainium2 Bass kernel for the VQ-codebook clustering model.

Computes, for x [131072, 784] fp32 and centers [64, 784] fp32:
    logits = 20 * (x @ centers.T - 0.5 * ||centers||^2)
    w      = softmax(logits, axis=1)
    recon  = w @ centers
and returns (recon, x) exactly like the reference (x0 == x here since x is
already 2-D, so it is passed through on the host).

Sharding: pure data parallel — x is split along the batch dim into 8 equal
shards of 16384 rows, centers are replicated; each NeuronCore runs the same
program on its shard and the host concatenates the outputs.

Per-core structure (macro-tile = 512 rows, super-tile = 4 macro-tiles):
  - DMA x in a (p g) layout: partition p holds 4 *consecutive* rows, so each
    partition's DMA piece is 12.5 KB contiguous (the within-group row
    permutation is honored symmetrically by the output store).  Two extra
    ones-columns feed the augmented bias rows.
  - Phase 1 (per super-tile): PE identity-transposes x into 7 feature chunks
    [112/114, 512] in f32r transpose mode; each psum chunk is evicted by a
    DVE half + ACT half in parallel into float32r SBUF tiles.
  - Phase 2 (per super-tile, software-pipelined): mm1 in float32r
    (logitsT [64,512] = sum_c ct_chunk.T @ xT_chunk, stationary = tiny
    centers chunk, 512-wide moving stream); the softmax/mm2 block of
    macro m-1 is emitted between mm1(m) chunks so the PE stream stays dense
    and the HAM clock gate stays at 2.4 GHz.  Chunk 6 carries hi/lo split
    augmented rows of -0.5*SCALE*||c||^2 so FP22 cannot bite the bias.
  - Softmax per 128-row group: PE transposes logitsT back (fp32, exact),
    DVE reduce-max (negated), ACT Exp with per-row bias + fused row-sum,
    DVE reciprocal; PE transposes e; mm2 in float32r against centers in
    natural layout; recon evicted DVE-half/ACT-half with 1/Z folded in.
"""

from contextlib import ExitStack

import numpy as np

import concourse.bass as bass
import concourse.tile as tile
import concourse.mybir as mybir
from concourse import bacc, masks
from concourse.bass_utils import run_bass_kernel_spmd

F32 = mybir.dt.float32
F32R = mybir.dt.float32r
BF16 = mybir.dt.bfloat16

N_CORES = 8
N_ROWS = 131072
D = 784
K = 64
SCALE = 20.0
ROWS_PER_CORE = N_ROWS // N_CORES  # 16384

GROUP = 128                  # rows per psum group (psum partition count)
GROUPS_PER_TILE = 4          # groups per macro tile
TILE_ROWS = GROUP * GROUPS_PER_TILE  # 512
SUPER = 6                    # macro-tiles per phase-batched super-tile
CHUNK = 112                  # feature-chunk width for the contraction
N_CHUNKS = D // CHUNK        # 7
NONES = 2                    # ones columns feeding the two augmented rows
EVICT_SPLIT = 256            # DVE evicts cols [0:256], ACT [256:512]
REC_SPLIT = 384              # recon evict: DVE [0:384], ACT [384:784]


def _pp(c):
    return CHUNK + NONES if c == N_CHUNKS - 1 else CHUNK


def emit_core_program(ctx: ExitStack, tc: tile.TileContext, x_ap, c_ap, y_ap,
                      rows_per_core):
    nc = tc.nc
    n_tiles = rows_per_core // TILE_ROWS

    const = ctx.enter_context(tc.tile_pool(name="const", bufs=1))
    xin_pool = ctx.enter_context(tc.tile_pool(name="xin", bufs=5))
    yout_pool = ctx.enter_context(tc.tile_pool(name="yout", bufs=2))
    xtsb_pool = ctx.enter_context(tc.tile_pool(name="xtsb",
                                               bufs=max(18, SUPER * N_CHUNKS + 4)))
    lt_pool = ctx.enter_context(tc.tile_pool(name="ltsb", bufs=2))
    e_pool = ctx.enter_context(tc.tile_pool(name="epool", bufs=2))
    etsb_pool = ctx.enter_context(tc.tile_pool(name="etsb", bufs=3))
    small_pool = ctx.enter_context(tc.tile_pool(name="small", bufs=4))

    xtps_pool = ctx.enter_context(tc.tile_pool(name="xtps", bufs=2, space="PSUM"))
    ltps_pool = ctx.enter_context(tc.tile_pool(name="ltps", bufs=2, space="PSUM"))
    lg_pool = ctx.enter_context(tc.tile_pool(name="lgps", bufs=1, space="PSUM"))
    etps_pool = ctx.enter_context(tc.tile_pool(name="etps", bufs=1, space="PSUM"))
    rec_pool = ctx.enter_context(tc.tile_pool(name="recps", bufs=1, space="PSUM"))

    # ---- preamble ----------------------------------------------------------
    ident = const.tile([128, 128], F32, tag="ident")
    masks.make_identity(nc, ident[:])
    ident_r = const.tile([128, 128], F32R, tag="identr")
    nc.vector.tensor_copy(ident_r[:], ident[:])

    cen = const.tile([K, D], F32, tag="cen")
    nc.sync.dma_start(out=cen[:], in_=c_ap[:, :])
    # second copy tagged float32r (same bytes) for the fp32r reconstruction
    # matmul — the BIR verifier wants fp32r operands produced as fp32r, and
    # only SWDGE DMA may change the dtype tag.
    cen_r = const.tile([K, D], F32R, tag="cenr")
    nc.gpsimd.dma_start(out=cen_r[:], in_=c_ap[:, :])

    # a = -0.5 * ||c||^2 per center, split into bf16 hi + fp32 lo so the
    # fp32r (FP22) read of the augmented rows cannot lose bias precision.
    sq_scratch = const.tile([K, D], F32, tag="sqscr")
    ssq = const.tile([K, 1], F32, tag="ssq")
    nc.scalar.activation(sq_scratch[:], cen[:],
                         mybir.ActivationFunctionType.Square,
                         accum_out=ssq[:])
    a_full = const.tile([K, 1], F32, tag="afull")
    nc.vector.tensor_scalar_mul(a_full[:], ssq[:], -0.5)
    a_hi16 = const.tile([K, 1], BF16, tag="ahi16")
    nc.vector.tensor_copy(a_hi16[:], a_full[:])
    a_hi = const.tile([K, 1], F32, tag="ahi")
    nc.vector.tensor_copy(a_hi[:], a_hi16[:])
    a_lo = const.tile([K, 1], F32, tag="alo")
    nc.vector.tensor_sub(a_lo[:], a_full[:], a_hi[:])

    # ct[:, 64c:64c+64] = chunk c of (SCALE * centers.T) as float32r;
    # partition rows 112/113 of chunk 6 are the augmented hi/lo bias rows.
    ct = const.tile([CHUNK + NONES, K * N_CHUNKS], F32R, tag="ct")
    for c in range(N_CHUNKS - 1):
        pre_ps = xtps_pool.tile([CHUNK, K], F32, tag="xtps")
        nc.tensor.transpose(out=pre_ps[0:CHUNK, 0:K],
                            in_=cen[:, c * CHUNK:(c + 1) * CHUNK],
                            identity=ident[0:K, 0:K])
        nc.scalar.mul(ct[0:CHUNK, c * K:(c + 1) * K], pre_ps[0:CHUNK, 0:K], SCALE)
    scr6 = const.tile([K, CHUNK + NONES], F32, tag="scr6")
    nc.vector.tensor_copy(scr6[:, 0:CHUNK],
                          cen[:, (N_CHUNKS - 1) * CHUNK:N_CHUNKS * CHUNK])
    nc.vector.tensor_copy(scr6[:, CHUNK:CHUNK + 1], a_hi[:])
    nc.vector.tensor_copy(scr6[:, CHUNK + 1:CHUNK + 2], a_lo[:])
    pre_ps6 = xtps_pool.tile([CHUNK + NONES, K], F32, tag="xtps")
    nc.tensor.transpose(out=pre_ps6[0:CHUNK + NONES, 0:K], in_=scr6[:],
                        identity=ident[0:K, 0:K])
    nc.scalar.mul(ct[0:CHUNK + NONES, (N_CHUNKS - 1) * K:N_CHUNKS * K],
                  pre_ps6[0:CHUNK + NONES, 0:K], SCALE)

    # ---- helpers -----------------------------------------------------------
    def emit_load_and_transpose(t):
        """Phase-1 body for macro-tile t: load + 28 transposes + evictions."""
        x_in = xin_pool.tile([128, GROUPS_PER_TILE, D + NONES], F32R, tag="xin")
        nc.sync.dma_start(
            out=x_in[:, :, 0:D],
            in_=x_ap[t * TILE_ROWS:(t + 1) * TILE_ROWS, :].rearrange(
                "(p g) f -> p g f", g=GROUPS_PER_TILE),
        )
        nc.gpsimd.memset(x_in[:, :, D:D + NONES].bitcast(mybir.dt.uint32),
                         0x3F800000)
        xt_sb = []
        for c in range(N_CHUNKS):
            pp = _pp(c)
            xt_ps = xtps_pool.tile([CHUNK + NONES, TILE_ROWS], F32R, tag="xtps")
            for g in range(GROUPS_PER_TILE):
                nc.tensor.transpose(
                    out=xt_ps[0:pp, g * GROUP:(g + 1) * GROUP],
                    in_=x_in[:, g, c * CHUNK:c * CHUNK + pp],
                    identity=ident_r[:, 0:GROUP])
            sb = xtsb_pool.tile([CHUNK + NONES, TILE_ROWS], F32R, tag="xtsb")
            nc.vector.tensor_copy(sb[0:pp, 0:EVICT_SPLIT],
                                  xt_ps[0:pp, 0:EVICT_SPLIT])
            nc.scalar.copy(sb[0:pp, EVICT_SPLIT:TILE_ROWS],
                           xt_ps[0:pp, EVICT_SPLIT:TILE_ROWS])
            xt_sb.append(sb)
        return xt_sb

    def emit_mm1(xt_sb):
        """Logits-transposed accumulation for one macro-tile."""
        lt_ps = ltps_pool.tile([K, TILE_ROWS], F32, tag="ltps")
        for c in range(N_CHUNKS):
            pp = _pp(c)
            nc.tensor.matmul(out=lt_ps[:, :],
                             lhsT=ct[0:pp, c * K:(c + 1) * K],
                             rhs=xt_sb[c][0:pp, :],
                             start=(c == 0), stop=(c == N_CHUNKS - 1))
        lt_sb = lt_pool.tile([K, TILE_ROWS], F32, tag="ltsb")
        nc.vector.tensor_copy(lt_sb[:, 0:EVICT_SPLIT], lt_ps[:, 0:EVICT_SPLIT])
        nc.scalar.copy(lt_sb[:, EVICT_SPLIT:TILE_ROWS],
                       lt_ps[:, EVICT_SPLIT:TILE_ROWS])
        return lt_sb

    def emit_softmax_mm2(t, lt_sb):
        """Softmax + reconstruction + store for one macro-tile."""
        out_sb = yout_pool.tile([128, GROUPS_PER_TILE, D], F32, tag="yout")
        for g in range(GROUPS_PER_TILE):
            lg_ps = lg_pool.tile([GROUP, K], F32, tag="lgps")
            nc.tensor.transpose(out=lg_ps[:, :],
                                in_=lt_sb[:, g * GROUP:(g + 1) * GROUP],
                                identity=ident[0:K, 0:K])

            negmax = small_pool.tile([GROUP, 1], F32, tag="negmax")
            nc.vector.tensor_reduce(out=negmax[:], in_=lg_ps[:, :],
                                    axis=mybir.AxisListType.X,
                                    op=mybir.AluOpType.max, negate=True)
            e_sb = e_pool.tile([GROUP, K], F32R, tag="esb")
            zsum = small_pool.tile([GROUP, 1], F32, tag="zsum")
            nc.scalar.activation(e_sb[:], lg_ps[:, :],
                                 mybir.ActivationFunctionType.Exp,
                                 bias=negmax[:], scale=1.0,
                                 accum_out=zsum[:])
            rinv = small_pool.tile([GROUP, 1], F32, tag="rinv")
            nc.vector.reciprocal(rinv[:], zsum[:])

            et_ps = etps_pool.tile([K, GROUP], F32R, tag="etps")
            nc.tensor.transpose(out=et_ps[0:K, :], in_=e_sb[:, :],
                                identity=ident_r[:, 0:GROUP])
            et_sb = etsb_pool.tile([K, GROUP], F32R, tag="etsb")
            nc.scalar.copy(et_sb[:], et_ps[0:K, :])

            rec_ps = rec_pool.tile([GROUP, D], F32, tag="recps")
            nc.tensor.matmul(out=rec_ps[:, 0:512], lhsT=et_sb[:],
                             rhs=cen_r[:, 0:512], start=True, stop=True)
            nc.tensor.matmul(out=rec_ps[:, 512:D], lhsT=et_sb[:],
                             rhs=cen_r[:, 512:D], start=True, stop=True)

            # evict with 1/Z normalization folded in, split DVE/ACT
            nc.vector.tensor_scalar_mul(out_sb[:, g, 0:REC_SPLIT],
                                        rec_ps[:, 0:REC_SPLIT], rinv[:])
            nc.scalar.mul(out_sb[:, g, REC_SPLIT:D],
                          rec_ps[:, REC_SPLIT:D], rinv[:])

        nc.sync.dma_start(
            out=y_ap[t * TILE_ROWS:(t + 1) * TILE_ROWS, :].rearrange(
                "(p g) f -> p g f", g=GROUPS_PER_TILE),
            in_=out_sb[:],
        )

    # ---- main loop: phase-batched super-tiles ------------------------------
    pending = None  # (t, lt_sb) global software pipeline
    for t0 in range(0, n_tiles, SUPER):
        ts = list(range(t0, min(t0 + SUPER, n_tiles)))
        xts = [emit_load_and_transpose(t) for t in ts]
        for i, t in enumerate(ts):
            lt_sb = emit_mm1(xts[i])
            if pending is not None:
                emit_softmax_mm2(*pending)
            pending = (t, lt_sb)
    emit_softmax_mm2(*pending)


def build_kernel(rows_per_core=ROWS_PER_CORE):
    nc = bacc.Bacc("TRN2", target_bir_lowering=False, debug=False)
    x_d = nc.dram_tensor("x", [rows_per_core, D], F32R, kind="ExternalInput")
    c_d = nc.dram_tensor("centers", [K, D], F32, kind="ExternalInput")
    y_d = nc.dram_tensor("y", [rows_per_core, D], F32, kind="ExternalOutput")
    with tile.TileContext(nc) as tc:
        with ExitStack() as ctx:
            emit_core_program(ctx, tc, x_d.ap(), c_d.ap(), y_ap=y_d.ap(),
                              rows_per_core=rows_per_core)
    nc.compile()
    return nc


_NC_CACHE = {}


def _get_nc(rows_per_core=ROWS_PER_CORE):
    if rows_per_core not in _NC_CACHE:
        _NC_CACHE[rows_per_core] = build_kernel(rows_per_core)
    return _NC_CACHE[rows_per_core]


def run_on_cores(x, centers, trace=False, **kwargs):
    """Run the SPMD kernel on 8 cores; returns (recon, BassKernelResults)."""
    x = np.ascontiguousarray(x, dtype=np.float32)
    centers = np.ascontiguousarray(centers, dtype=np.float32)
    assert x.shape == (N_ROWS, D) and centers.shape == (K, D)
    nc = _get_nc()
    shards = x.reshape(N_CORES, ROWS_PER_CORE, D)
    in_maps = [{"x": shards[i], "centers": centers} for i in range(N_CORES)]
    br = run_bass_kernel_spmd(nc, in_maps, list(range(N_CORES)), trace=trace,
                              **kwargs)
    recon = np.concatenate([r["y"] for r in br.results], axis=0)
    return recon, br


def kernel(x, centers):
    x = np.ascontiguousarray(x, dtype=np.float32)
    recon, _ = run_on_cores(x, centers)
    return recon, x


# revision 20
# speedup vs baseline: 1.1875x; 1.0262x over previous
"""Trainium2 Bass kernel for the VQ-codebook clustering model.

Computes, for x [131072, 784] fp32 and centers [64, 784] fp32:
    logits = 20 * (x @ centers.T - 0.5 * ||centers||^2)
    w      = softmax(logits, axis=1)
    recon  = w @ centers
and returns (recon, x) exactly like the reference (x0 == x here since x is
already 2-D, so it is passed through on the host).

Sharding: pure data parallel — x is split along the batch dim into 8 equal
shards of 16384 rows, centers are replicated; each NeuronCore runs the same
program on its shard and the host concatenates the outputs.

Per-core structure (macro-tile = 512 rows, super-tile = 4 macro-tiles):
  - DMA x in a (p g) layout: partition p holds 4 *consecutive* rows, so each
    partition's DMA piece is 12.5 KB contiguous (the within-group row
    permutation is honored symmetrically by the output store).  Two extra
    ones-columns feed the augmented bias rows.
  - Phase 1 (per super-tile): PE identity-transposes x into 7 feature chunks
    [112/114, 512] in f32r transpose mode; each psum chunk is evicted by a
    DVE half + ACT half in parallel into float32r SBUF tiles.
  - Phase 2 (per super-tile, software-pipelined): mm1 in float32r
    (logitsT [64,512] = sum_c ct_chunk.T @ xT_chunk, stationary = tiny
    centers chunk, 512-wide moving stream); the softmax/mm2 block of
    macro m-1 is emitted between mm1(m) chunks so the PE stream stays dense
    and the HAM clock gate stays at 2.4 GHz.  Chunk 6 carries hi/lo split
    augmented rows of -0.5*SCALE*||c||^2 so FP22 cannot bite the bias.
  - Softmax per 128-row group: PE transposes logitsT back (fp32, exact),
    DVE reduce-max (negated), ACT Exp with per-row bias + fused row-sum,
    DVE reciprocal; PE transposes e; mm2 in float32r against centers in
    natural layout; recon evicted DVE-half/ACT-half with 1/Z folded in.
"""

from contextlib import ExitStack

import numpy as np

import concourse.bass as bass
import concourse.tile as tile
import concourse.mybir as mybir
from concourse import bacc, masks
from concourse.bass_utils import run_bass_kernel_spmd

F32 = mybir.dt.float32
F32R = mybir.dt.float32r
BF16 = mybir.dt.bfloat16

N_CORES = 8
N_ROWS = 131072
D = 784
K = 64
SCALE = 20.0
ROWS_PER_CORE = N_ROWS // N_CORES  # 16384

GROUP = 128                  # rows per psum group (psum partition count)
GROUPS_PER_TILE = 4          # groups per macro tile
TILE_ROWS = GROUP * GROUPS_PER_TILE  # 512
SUPER = 6                    # macro-tiles per phase-batched super-tile
CHUNK = 112                  # feature-chunk width for the contraction
N_CHUNKS = D // CHUNK        # 7
NONES = 2                    # ones columns feeding the two augmented rows
EVICT_SPLIT = 256            # DVE evicts cols [0:256], ACT [256:512]
REC_SPLIT = 384              # recon evict: DVE [0:384], ACT [384:784]


def _pp(c):
    return CHUNK + NONES if c == N_CHUNKS - 1 else CHUNK


def emit_core_program(ctx: ExitStack, tc: tile.TileContext, x_ap, c_ap, y_ap,
                      rows_per_core):
    nc = tc.nc
    n_tiles = rows_per_core // TILE_ROWS

    const = ctx.enter_context(tc.tile_pool(name="const", bufs=1))
    xin_pool = ctx.enter_context(tc.tile_pool(name="xin", bufs=5))
    yout_pool = ctx.enter_context(tc.tile_pool(name="yout", bufs=2))
    xtsb_pool = ctx.enter_context(tc.tile_pool(name="xtsb",
                                               bufs=max(18, SUPER * N_CHUNKS + 4)))
    lt_pool = ctx.enter_context(tc.tile_pool(name="ltsb", bufs=2))
    e_pool = ctx.enter_context(tc.tile_pool(name="epool", bufs=2))
    etsb_pool = ctx.enter_context(tc.tile_pool(name="etsb", bufs=5))
    small_pool = ctx.enter_context(tc.tile_pool(name="small", bufs=6))

    xtps_pool = ctx.enter_context(tc.tile_pool(name="xtps", bufs=2, space="PSUM"))
    ltps_pool = ctx.enter_context(tc.tile_pool(name="ltps", bufs=1, space="PSUM"))
    soft_pool = ctx.enter_context(tc.tile_pool(name="softps", bufs=3, space="PSUM"))
    rec_pool = ctx.enter_context(tc.tile_pool(name="recps", bufs=1, space="PSUM"))

    # ---- preamble ----------------------------------------------------------
    ident = const.tile([128, 128], F32, tag="ident")
    masks.make_identity(nc, ident[:])
    ident_r = const.tile([128, 128], F32R, tag="identr")
    nc.vector.tensor_copy(ident_r[:], ident[:])

    cen = const.tile([K, D], F32, tag="cen")
    nc.sync.dma_start(out=cen[:], in_=c_ap[:, :])
    # second copy tagged float32r (same bytes) for the fp32r reconstruction
    # matmul — the BIR verifier wants fp32r operands produced as fp32r, and
    # only SWDGE DMA may change the dtype tag.
    cen_r = const.tile([K, D], F32R, tag="cenr")
    nc.gpsimd.dma_start(out=cen_r[:], in_=c_ap[:, :])

    # a = -0.5 * ||c||^2 per center, split into bf16 hi + fp32 lo so the
    # fp32r (FP22) read of the augmented rows cannot lose bias precision.
    sq_scratch = const.tile([K, D], F32, tag="sqscr")
    ssq = const.tile([K, 1], F32, tag="ssq")
    nc.scalar.activation(sq_scratch[:], cen[:],
                         mybir.ActivationFunctionType.Square,
                         accum_out=ssq[:])
    a_full = const.tile([K, 1], F32, tag="afull")
    nc.vector.tensor_scalar_mul(a_full[:], ssq[:], -0.5)
    a_hi16 = const.tile([K, 1], BF16, tag="ahi16")
    nc.vector.tensor_copy(a_hi16[:], a_full[:])
    a_hi = const.tile([K, 1], F32, tag="ahi")
    nc.vector.tensor_copy(a_hi[:], a_hi16[:])
    a_lo = const.tile([K, 1], F32, tag="alo")
    nc.vector.tensor_sub(a_lo[:], a_full[:], a_hi[:])

    # ct[:, 64c:64c+64] = chunk c of (SCALE * centers.T) as float32r;
    # partition rows 112/113 of chunk 6 are the augmented hi/lo bias rows.
    ct = const.tile([CHUNK + NONES, K * N_CHUNKS], F32R, tag="ct")
    for c in range(N_CHUNKS - 1):
        pre_ps = xtps_pool.tile([CHUNK, K], F32, tag="xtps")
        nc.tensor.transpose(out=pre_ps[0:CHUNK, 0:K],
                            in_=cen[:, c * CHUNK:(c + 1) * CHUNK],
                            identity=ident[0:K, 0:K])
        nc.scalar.mul(ct[0:CHUNK, c * K:(c + 1) * K], pre_ps[0:CHUNK, 0:K], SCALE)
    scr6 = const.tile([K, CHUNK + NONES], F32, tag="scr6")
    nc.vector.tensor_copy(scr6[:, 0:CHUNK],
                          cen[:, (N_CHUNKS - 1) * CHUNK:N_CHUNKS * CHUNK])
    nc.vector.tensor_copy(scr6[:, CHUNK:CHUNK + 1], a_hi[:])
    nc.vector.tensor_copy(scr6[:, CHUNK + 1:CHUNK + 2], a_lo[:])
    pre_ps6 = xtps_pool.tile([CHUNK + NONES, K], F32, tag="xtps")
    nc.tensor.transpose(out=pre_ps6[0:CHUNK + NONES, 0:K], in_=scr6[:],
                        identity=ident[0:K, 0:K])
    nc.scalar.mul(ct[0:CHUNK + NONES, (N_CHUNKS - 1) * K:N_CHUNKS * K],
                  pre_ps6[0:CHUNK + NONES, 0:K], SCALE)

    # ---- helpers -----------------------------------------------------------
    def emit_load_and_transpose(t):
        """Phase-1 body for macro-tile t: load + 28 transposes + evictions."""
        x_in = xin_pool.tile([128, GROUPS_PER_TILE, D + NONES], F32R, tag="xin")
        nc.sync.dma_start(
            out=x_in[:, :, 0:D],
            in_=x_ap[t * TILE_ROWS:(t + 1) * TILE_ROWS, :].rearrange(
                "(p g) f -> p g f", g=GROUPS_PER_TILE),
        )
        nc.gpsimd.memset(x_in[:, :, D:D + NONES].bitcast(mybir.dt.uint32),
                         0x3F800000)
        xt_sb = []
        for c in range(N_CHUNKS):
            pp = _pp(c)
            xt_ps = xtps_pool.tile([CHUNK + NONES, TILE_ROWS], F32R, tag="xtps")
            for g in range(GROUPS_PER_TILE):
                nc.tensor.transpose(
                    out=xt_ps[0:pp, g * GROUP:(g + 1) * GROUP],
                    in_=x_in[:, g, c * CHUNK:c * CHUNK + pp],
                    identity=ident_r[:, 0:GROUP])
            sb = xtsb_pool.tile([CHUNK + NONES, TILE_ROWS], F32R, tag="xtsb")
            nc.vector.tensor_copy(sb[0:pp, 0:EVICT_SPLIT],
                                  xt_ps[0:pp, 0:EVICT_SPLIT])
            nc.scalar.copy(sb[0:pp, EVICT_SPLIT:TILE_ROWS],
                           xt_ps[0:pp, EVICT_SPLIT:TILE_ROWS])
            xt_sb.append(sb)
        return xt_sb

    def emit_mm1(xt_sb):
        """Logits-transposed accumulation for one macro-tile."""
        lt_ps = ltps_pool.tile([K, TILE_ROWS], F32, tag="ltps")
        for c in range(N_CHUNKS):
            pp = _pp(c)
            nc.tensor.matmul(out=lt_ps[:, :],
                             lhsT=ct[0:pp, c * K:(c + 1) * K],
                             rhs=xt_sb[c][0:pp, :],
                             start=(c == 0), stop=(c == N_CHUNKS - 1))
        lt_sb = lt_pool.tile([K, TILE_ROWS], F32, tag="ltsb")
        nc.vector.tensor_copy(lt_sb[:, 0:EVICT_SPLIT], lt_ps[:, 0:EVICT_SPLIT])
        nc.scalar.copy(lt_sb[:, EVICT_SPLIT:TILE_ROWS],
                       lt_ps[:, EVICT_SPLIT:TILE_ROWS])
        return lt_sb

    def emit_softmax_mm2(t, lt_sb):
        """Softmax (all groups) then reconstruction + store for one tile.

        All four groups' eT operands are produced before the first mm2 so
        the fp32r fused weight loads never wait on the ACT eviction chain.
        """
        out_sb = yout_pool.tile([128, GROUPS_PER_TILE, D], F32, tag="yout")
        ets, rinvs = [], []
        for g in range(GROUPS_PER_TILE):
            lg_ps = soft_pool.tile([GROUP, K], F32, tag="softps")
            nc.tensor.transpose(out=lg_ps[:, :],
                                in_=lt_sb[:, g * GROUP:(g + 1) * GROUP],
                                identity=ident[0:K, 0:K])

            negmax = small_pool.tile([GROUP, 1], F32, tag="negmax")
            nc.vector.tensor_reduce(out=negmax[:], in_=lg_ps[:, :],
                                    axis=mybir.AxisListType.X,
                                    op=mybir.AluOpType.max, negate=True)
            e_sb = e_pool.tile([GROUP, K], F32R, tag="esb")
            zsum = small_pool.tile([GROUP, 1], F32, tag="zsum")
            nc.scalar.activation(e_sb[:], lg_ps[:, :],
                                 mybir.ActivationFunctionType.Exp,
                                 bias=negmax[:], scale=1.0,
                                 accum_out=zsum[:])
            rinv = small_pool.tile([GROUP, 1], F32, tag="rinv")
            nc.vector.reciprocal(rinv[:], zsum[:])

            et_ps = soft_pool.tile([K, GROUP], F32R, tag="softps")
            nc.tensor.transpose(out=et_ps[0:K, :], in_=e_sb[:, :],
                                identity=ident_r[:, 0:GROUP])
            et_sb = etsb_pool.tile([K, GROUP], F32R, tag="etsb")
            nc.scalar.copy(et_sb[:], et_ps[0:K, :])
            ets.append(et_sb)
            rinvs.append(rinv)

        for g in range(GROUPS_PER_TILE):
            rec_ps = rec_pool.tile([GROUP, D], F32, tag="recps")
            nc.tensor.matmul(out=rec_ps[:, 0:512], lhsT=ets[g][:],
                             rhs=cen_r[:, 0:512], start=True, stop=True)
            nc.tensor.matmul(out=rec_ps[:, 512:D], lhsT=ets[g][:],
                             rhs=cen_r[:, 512:D], start=True, stop=True)

            # evict with 1/Z normalization folded in, split DVE/ACT
            nc.vector.tensor_scalar_mul(out_sb[:, g, 0:REC_SPLIT],
                                        rec_ps[:, 0:REC_SPLIT], rinvs[g][:]),
            nc.scalar.mul(out_sb[:, g, REC_SPLIT:D],
                          rec_ps[:, REC_SPLIT:D], rinvs[g][:])

        nc.sync.dma_start(
            out=y_ap[t * TILE_ROWS:(t + 1) * TILE_ROWS, :].rearrange(
                "(p g) f -> p g f", g=GROUPS_PER_TILE),
            in_=out_sb[:],
        )

    # ---- main loop: phase-batched super-tiles ------------------------------
    pending = None  # (t, lt_sb) global software pipeline
    for t0 in range(0, n_tiles, SUPER):
        ts = list(range(t0, min(t0 + SUPER, n_tiles)))
        xts = [emit_load_and_transpose(t) for t in ts]
        for i, t in enumerate(ts):
            lt_sb = emit_mm1(xts[i])
            if pending is not None:
                emit_softmax_mm2(*pending)
            pending = (t, lt_sb)
    emit_softmax_mm2(*pending)


def build_kernel(rows_per_core=ROWS_PER_CORE):
    nc = bacc.Bacc("TRN2", target_bir_lowering=False, debug=False)
    x_d = nc.dram_tensor("x", [rows_per_core, D], F32R, kind="ExternalInput")
    c_d = nc.dram_tensor("centers", [K, D], F32, kind="ExternalInput")
    y_d = nc.dram_tensor("y", [rows_per_core, D], F32, kind="ExternalOutput")
    with tile.TileContext(nc) as tc:
        with ExitStack() as ctx:
            emit_core_program(ctx, tc, x_d.ap(), c_d.ap(), y_ap=y_d.ap(),
                              rows_per_core=rows_per_core)
    nc.compile()
    return nc


_NC_CACHE = {}


def _get_nc(rows_per_core=ROWS_PER_CORE):
    if rows_per_core not in _NC_CACHE:
        _NC_CACHE[rows_per_core] = build_kernel(rows_per_core)
    return _NC_CACHE[rows_per_core]


def run_on_cores(x, centers, trace=False, **kwargs):
    """Run the SPMD kernel on 8 cores; returns (recon, BassKernelResults)."""
    x = np.ascontiguousarray(x, dtype=np.float32)
    centers = np.ascontiguousarray(centers, dtype=np.float32)
    assert x.shape == (N_ROWS, D) and centers.shape == (K, D)
    nc = _get_nc()
    shards = x.reshape(N_CORES, ROWS_PER_CORE, D)
    in_maps = [{"x": shards[i], "centers": centers} for i in range(N_CORES)]
    br = run_bass_kernel_spmd(nc, in_maps, list(range(N_CORES)), trace=trace,
                              **kwargs)
    recon = np.concatenate([r["y"] for r in br.results], axis=0)
    return recon, br


def kernel(x, centers):
    x = np.ascontiguousarray(x, dtype=np.float32)
    recon, _ = run_on_cores(x, centers)
    return recon, x


# revision 21
# speedup vs baseline: 1.2120x; 1.0206x over previous
"""Trainium2 Bass kernel for the VQ-codebook clustering model.

Computes, for x [131072, 784] fp32 and centers [64, 784] fp32:
    logits = 20 * (x @ centers.T - 0.5 * ||centers||^2)
    w      = softmax(logits, axis=1)
    recon  = w @ centers
and returns (recon, x) exactly like the reference (x0 == x here since x is
already 2-D, so it is passed through on the host).

Sharding: pure data parallel — x is split along the batch dim into 8 equal
shards of 16384 rows, centers are replicated; each NeuronCore runs the same
program on its shard and the host concatenates the outputs.

Per-core structure (macro-tile = 512 rows, super-tile = 4 macro-tiles):
  - DMA x in a (p g) layout: partition p holds 4 *consecutive* rows, so each
    partition's DMA piece is 12.5 KB contiguous (the within-group row
    permutation is honored symmetrically by the output store).  Two extra
    ones-columns feed the augmented bias rows.
  - Phase 1 (per super-tile): PE identity-transposes x into 7 feature chunks
    [112/114, 512] in f32r transpose mode; each psum chunk is evicted by a
    DVE half + ACT half in parallel into float32r SBUF tiles.
  - Phase 2 (per super-tile, software-pipelined): mm1 in float32r
    (logitsT [64,512] = sum_c ct_chunk.T @ xT_chunk, stationary = tiny
    centers chunk, 512-wide moving stream); the softmax/mm2 block of
    macro m-1 is emitted between mm1(m) chunks so the PE stream stays dense
    and the HAM clock gate stays at 2.4 GHz.  Chunk 6 carries hi/lo split
    augmented rows of -0.5*SCALE*||c||^2 so FP22 cannot bite the bias.
  - Softmax per 128-row group: PE transposes logitsT back (fp32, exact),
    DVE reduce-max (negated), ACT Exp with per-row bias + fused row-sum,
    DVE reciprocal; PE transposes e; mm2 in float32r against centers in
    natural layout; recon evicted DVE-half/ACT-half with 1/Z folded in.
"""

from contextlib import ExitStack

import numpy as np

import concourse.bass as bass
import concourse.tile as tile
import concourse.mybir as mybir
from concourse import bacc, masks
from concourse.bass_utils import run_bass_kernel_spmd

F32 = mybir.dt.float32
F32R = mybir.dt.float32r
BF16 = mybir.dt.bfloat16

N_CORES = 8
N_ROWS = 131072
D = 784
K = 64
SCALE = 20.0
ROWS_PER_CORE = N_ROWS // N_CORES  # 16384

GROUP = 128                  # rows per psum group (psum partition count)
GROUPS_PER_TILE = 4          # groups per macro tile
TILE_ROWS = GROUP * GROUPS_PER_TILE  # 512
SUPER = 6                    # macro-tiles per phase-batched super-tile
CHUNK = 112                  # feature-chunk width for the contraction
N_CHUNKS = D // CHUNK        # 7
NONES = 2                    # ones columns feeding the two augmented rows
EVICT_SPLIT = 256            # DVE evicts cols [0:256], ACT [256:512]
REC_SPLIT = 384              # recon evict: DVE [0:384], ACT [384:784]


def _pp(c):
    return CHUNK + NONES if c == N_CHUNKS - 1 else CHUNK


def emit_core_program(ctx: ExitStack, tc: tile.TileContext, x_ap, c_ap, y_ap,
                      rows_per_core):
    nc = tc.nc
    n_tiles = rows_per_core // TILE_ROWS

    const = ctx.enter_context(tc.tile_pool(name="const", bufs=1))
    xin_pool = ctx.enter_context(tc.tile_pool(name="xin", bufs=5))
    yout_pool = ctx.enter_context(tc.tile_pool(name="yout", bufs=2))
    xtsb_pool = ctx.enter_context(tc.tile_pool(name="xtsb",
                                               bufs=max(18, SUPER * N_CHUNKS + 4)))
    lt_pool = ctx.enter_context(tc.tile_pool(name="ltsb", bufs=2))
    e_pool = ctx.enter_context(tc.tile_pool(name="epool", bufs=2))
    etsb_pool = ctx.enter_context(tc.tile_pool(name="etsb", bufs=5))
    small_pool = ctx.enter_context(tc.tile_pool(name="small", bufs=6))

    xtps_pool = ctx.enter_context(tc.tile_pool(name="xtps", bufs=2, space="PSUM"))
    ltps_pool = ctx.enter_context(tc.tile_pool(name="ltps", bufs=1, space="PSUM"))
    soft_pool = ctx.enter_context(tc.tile_pool(name="softps", bufs=3, space="PSUM"))
    rec_pool = ctx.enter_context(tc.tile_pool(name="recps", bufs=1, space="PSUM"))

    # ---- preamble ----------------------------------------------------------
    ident = const.tile([128, 128], F32, tag="ident")
    masks.make_identity(nc, ident[:])
    ident_r = const.tile([128, 128], F32R, tag="identr")
    nc.vector.tensor_copy(ident_r[:], ident[:])

    cen = const.tile([K, D], F32, tag="cen")
    nc.sync.dma_start(out=cen[:], in_=c_ap[:, :])
    # second copy tagged float32r (same bytes) for the fp32r reconstruction
    # matmul — the BIR verifier wants fp32r operands produced as fp32r, and
    # only SWDGE DMA may change the dtype tag.
    cen_b = const.tile([K, D], BF16, tag="cenb")
    nc.gpsimd.dma_start(out=cen_b[:], in_=c_ap[:, :])

    # a = -0.5 * ||c||^2 per center, split into bf16 hi + fp32 lo so the
    # fp32r (FP22) read of the augmented rows cannot lose bias precision.
    sq_scratch = const.tile([K, D], F32, tag="sqscr")
    ssq = const.tile([K, 1], F32, tag="ssq")
    nc.scalar.activation(sq_scratch[:], cen[:],
                         mybir.ActivationFunctionType.Square,
                         accum_out=ssq[:])
    a_full = const.tile([K, 1], F32, tag="afull")
    nc.vector.tensor_scalar_mul(a_full[:], ssq[:], -0.5)
    a_hi16 = const.tile([K, 1], BF16, tag="ahi16")
    nc.vector.tensor_copy(a_hi16[:], a_full[:])
    a_hi = const.tile([K, 1], F32, tag="ahi")
    nc.vector.tensor_copy(a_hi[:], a_hi16[:])
    a_lo = const.tile([K, 1], F32, tag="alo")
    nc.vector.tensor_sub(a_lo[:], a_full[:], a_hi[:])

    # ct[:, 64c:64c+64] = chunk c of (SCALE * centers.T) as float32r;
    # partition rows 112/113 of chunk 6 are the augmented hi/lo bias rows.
    ct = const.tile([CHUNK + NONES, K * N_CHUNKS], F32R, tag="ct")
    for c in range(N_CHUNKS - 1):
        pre_ps = xtps_pool.tile([CHUNK, K], F32, tag="xtps")
        nc.tensor.transpose(out=pre_ps[0:CHUNK, 0:K],
                            in_=cen[:, c * CHUNK:(c + 1) * CHUNK],
                            identity=ident[0:K, 0:K])
        nc.scalar.mul(ct[0:CHUNK, c * K:(c + 1) * K], pre_ps[0:CHUNK, 0:K], SCALE)
    scr6 = const.tile([K, CHUNK + NONES], F32, tag="scr6")
    nc.vector.tensor_copy(scr6[:, 0:CHUNK],
                          cen[:, (N_CHUNKS - 1) * CHUNK:N_CHUNKS * CHUNK])
    nc.vector.tensor_copy(scr6[:, CHUNK:CHUNK + 1], a_hi[:])
    nc.vector.tensor_copy(scr6[:, CHUNK + 1:CHUNK + 2], a_lo[:])
    pre_ps6 = xtps_pool.tile([CHUNK + NONES, K], F32, tag="xtps")
    nc.tensor.transpose(out=pre_ps6[0:CHUNK + NONES, 0:K], in_=scr6[:],
                        identity=ident[0:K, 0:K])
    nc.scalar.mul(ct[0:CHUNK + NONES, (N_CHUNKS - 1) * K:N_CHUNKS * K],
                  pre_ps6[0:CHUNK + NONES, 0:K], SCALE)

    # ---- helpers -----------------------------------------------------------
    def emit_load_and_transpose(t):
        """Phase-1 body for macro-tile t: load + 28 transposes + evictions."""
        x_in = xin_pool.tile([128, GROUPS_PER_TILE, D + NONES], F32R, tag="xin")
        nc.sync.dma_start(
            out=x_in[:, :, 0:D],
            in_=x_ap[t * TILE_ROWS:(t + 1) * TILE_ROWS, :].rearrange(
                "(p g) f -> p g f", g=GROUPS_PER_TILE),
        )
        nc.gpsimd.memset(x_in[:, :, D:D + NONES].bitcast(mybir.dt.uint32),
                         0x3F800000)
        xt_sb = []
        for c in range(N_CHUNKS):
            pp = _pp(c)
            xt_ps = xtps_pool.tile([CHUNK + NONES, TILE_ROWS], F32R, tag="xtps")
            for g in range(GROUPS_PER_TILE):
                nc.tensor.transpose(
                    out=xt_ps[0:pp, g * GROUP:(g + 1) * GROUP],
                    in_=x_in[:, g, c * CHUNK:c * CHUNK + pp],
                    identity=ident_r[:, 0:GROUP])
            sb = xtsb_pool.tile([CHUNK + NONES, TILE_ROWS], F32R, tag="xtsb")
            nc.vector.tensor_copy(sb[0:pp, 0:EVICT_SPLIT],
                                  xt_ps[0:pp, 0:EVICT_SPLIT])
            nc.scalar.copy(sb[0:pp, EVICT_SPLIT:TILE_ROWS],
                           xt_ps[0:pp, EVICT_SPLIT:TILE_ROWS])
            xt_sb.append(sb)
        return xt_sb

    def emit_mm1(xt_sb):
        """Logits-transposed accumulation for one macro-tile."""
        lt_ps = ltps_pool.tile([K, TILE_ROWS], F32, tag="ltps")
        for c in range(N_CHUNKS):
            pp = _pp(c)
            nc.tensor.matmul(out=lt_ps[:, :],
                             lhsT=ct[0:pp, c * K:(c + 1) * K],
                             rhs=xt_sb[c][0:pp, :],
                             start=(c == 0), stop=(c == N_CHUNKS - 1))
        lt_sb = lt_pool.tile([K, TILE_ROWS], F32, tag="ltsb")
        nc.vector.tensor_copy(lt_sb[:, 0:EVICT_SPLIT], lt_ps[:, 0:EVICT_SPLIT])
        nc.scalar.copy(lt_sb[:, EVICT_SPLIT:TILE_ROWS],
                       lt_ps[:, EVICT_SPLIT:TILE_ROWS])
        return lt_sb

    def emit_softmax_mm2(t, lt_sb):
        """Softmax (all groups) then reconstruction + store for one tile.

        All four groups' eT operands are produced before the first mm2 so
        the fp32r fused weight loads never wait on the ACT eviction chain.
        """
        out_sb = yout_pool.tile([128, GROUPS_PER_TILE, D], F32, tag="yout")
        ets, rinvs = [], []
        for g in range(GROUPS_PER_TILE):
            lg_ps = soft_pool.tile([GROUP, K], F32, tag="softps")
            nc.tensor.transpose(out=lg_ps[:, :],
                                in_=lt_sb[:, g * GROUP:(g + 1) * GROUP],
                                identity=ident[0:K, 0:K])

            negmax = small_pool.tile([GROUP, 1], F32, tag="negmax")
            nc.vector.tensor_reduce(out=negmax[:], in_=lg_ps[:, :],
                                    axis=mybir.AxisListType.X,
                                    op=mybir.AluOpType.max, negate=True)
            e_sb = e_pool.tile([GROUP, K], F32R, tag="esb")
            zsum = small_pool.tile([GROUP, 1], F32, tag="zsum")
            nc.scalar.activation(e_sb[:], lg_ps[:, :],
                                 mybir.ActivationFunctionType.Exp,
                                 bias=negmax[:], scale=1.0,
                                 accum_out=zsum[:])
            rinv = small_pool.tile([GROUP, 1], F32, tag="rinv")
            nc.vector.reciprocal(rinv[:], zsum[:])

            et_ps = soft_pool.tile([K, GROUP], F32R, tag="softps")
            nc.tensor.transpose(out=et_ps[0:K, :], in_=e_sb[:, :],
                                identity=ident_r[:, 0:GROUP])
            et_sb = etsb_pool.tile([K, GROUP], BF16, tag="etsb")
            nc.vector.tensor_copy(et_sb[:], et_ps[0:K, :])
            ets.append(et_sb)
            rinvs.append(rinv)

        for g in range(GROUPS_PER_TILE):
            rec_ps = rec_pool.tile([GROUP, D], F32, tag="recps")
            nc.tensor.matmul(out=rec_ps[:, 0:512], lhsT=ets[g][:],
                             rhs=cen_b[:, 0:512], start=True, stop=True)
            nc.tensor.matmul(out=rec_ps[:, 512:D], lhsT=ets[g][:],
                             rhs=cen_b[:, 512:D], start=True, stop=True)

            # evict with 1/Z normalization folded in, split DVE/ACT
            nc.vector.tensor_scalar_mul(out_sb[:, g, 0:REC_SPLIT],
                                        rec_ps[:, 0:REC_SPLIT], rinvs[g][:]),
            nc.scalar.mul(out_sb[:, g, REC_SPLIT:D],
                          rec_ps[:, REC_SPLIT:D], rinvs[g][:])

        nc.sync.dma_start(
            out=y_ap[t * TILE_ROWS:(t + 1) * TILE_ROWS, :].rearrange(
                "(p g) f -> p g f", g=GROUPS_PER_TILE),
            in_=out_sb[:],
        )

    # ---- main loop: phase-batched super-tiles ------------------------------
    pending = None  # (t, lt_sb) global software pipeline
    for t0 in range(0, n_tiles, SUPER):
        ts = list(range(t0, min(t0 + SUPER, n_tiles)))
        xts = [emit_load_and_transpose(t) for t in ts]
        for i, t in enumerate(ts):
            lt_sb = emit_mm1(xts[i])
            if pending is not None:
                emit_softmax_mm2(*pending)
            pending = (t, lt_sb)
    emit_softmax_mm2(*pending)


def build_kernel(rows_per_core=ROWS_PER_CORE):
    nc = bacc.Bacc("TRN2", target_bir_lowering=False, debug=False)
    x_d = nc.dram_tensor("x", [rows_per_core, D], F32R, kind="ExternalInput")
    c_d = nc.dram_tensor("centers", [K, D], F32, kind="ExternalInput")
    y_d = nc.dram_tensor("y", [rows_per_core, D], F32, kind="ExternalOutput")
    with tile.TileContext(nc) as tc:
        with ExitStack() as ctx:
            emit_core_program(ctx, tc, x_d.ap(), c_d.ap(), y_ap=y_d.ap(),
                              rows_per_core=rows_per_core)
    nc.compile()
    return nc


_NC_CACHE = {}


def _get_nc(rows_per_core=ROWS_PER_CORE):
    if rows_per_core not in _NC_CACHE:
        _NC_CACHE[rows_per_core] = build_kernel(rows_per_core)
    return _NC_CACHE[rows_per_core]


def run_on_cores(x, centers, trace=False, **kwargs):
    """Run the SPMD kernel on 8 cores; returns (recon, BassKernelResults)."""
    x = np.ascontiguousarray(x, dtype=np.float32)
    centers = np.ascontiguousarray(centers, dtype=np.float32)
    assert x.shape == (N_ROWS, D) and centers.shape == (K, D)
    nc = _get_nc()
    shards = x.reshape(N_CORES, ROWS_PER_CORE, D)
    in_maps = [{"x": shards[i], "centers": centers} for i in range(N_CORES)]
    br = run_bass_kernel_spmd(nc, in_maps, list(range(N_CORES)), trace=trace,
                              **kwargs)
    recon = np.concatenate([r["y"] for r in br.results], axis=0)
    return recon, br


def kernel(x, centers):
    x = np.ascontiguousarray(x, dtype=np.float32)
    recon, _ = run_on_cores(x, centers)
    return recon, x
